# revision 15
# baseline (speedup 1.0000x reference)
"""BitLinear (binarized linear + activation-LN) Trainium2 kernel.

Full-input contract: kernel(x[8192,2048] f32, weight[2048,2048] f32,
bias[2048] f32) -> y[8192,2048] f32, data-parallel over 8 NeuronCores
(1024 rows of x per core; weight/bias replicated).

Math (gama cancels exactly between the activation quant scale and the
output dequant scale; the clip at +-(qb-eps) only perturbs the row max
by ~1e-6 relative):

    y[b,o] = beta_o * sum_i (r_b x[b,i]) sign(w[o,i]-mu_o)
             - beta_o mu_b r_b S[o] + bias[o]
    with S[o] = sum_i sign(w[o,i]-mu_o),  r_b = rsqrt(var_b + eps)

Layout: the matmul runs transposed, out[o-block, b] per 128-row weight
tile, so every o-indexed stat (beta, S, bias) is per-partition and the
whole pipeline for one weight tile (load -> stats -> sign -> xbar
transpose -> 32 matmuls -> fused epilogue) is independent of the other
15. The b-indexed stats (-mu*r rows) are tiny and bounced through DRAM
once. The kernel writes y^T; the host un-transposes (layout only).
"""

import os

import numpy as np

import concourse.bass as bass
import concourse.mybir as mybir
import concourse.tile as tile
from concourse import bacc
from concourse.bass_utils import run_bass_kernel_spmd

N_CORES = 8
BATCH = 8192
IN_F = 2048
OUT_F = 2048
B = BATCH // N_CORES  # rows of x per core
P = 128
KT = IN_F // P   # contraction blocks
OT = OUT_F // P  # weight row tiles (= output psum groups)
BT = B // P      # x row tiles per core
NC_CHUNK = 512   # matmul moving free dim
BC = B // NC_CHUNK
EPS = 1e-5

F16 = mybir.dt.float16
F32 = mybir.dt.float32
MUL = mybir.AluOpType.mult
ADD = mybir.AluOpType.add
AXF = mybir.AxisListType.X
AF = mybir.ActivationFunctionType


def _build_program() -> bass.Bass:
    nc = bacc.Bacc("TRN2", target_bir_lowering=False, debug=False)

    x16_h = nc.dram_tensor("x16", [B, IN_F], F16, kind="ExternalInput")
    w_h = nc.dram_tensor("w", [OUT_F, IN_F], F32, kind="ExternalInput")
    bias_h = nc.dram_tensor("bias16", [1, OUT_F], F16, kind="ExternalInput")
    yT_h = nc.dram_tensor("yT16", [OUT_F, B], F16, kind="ExternalOutput")
    # tiny DRAM bounces for partition->free row rearrangement
    nmr_h = nc.dram_tensor("nmr_d", [1, B], F16)
    r_h = nc.dram_tensor("r_d", [1, B], F16)

    x16 = x16_h[:, :]
    w = w_h[:, :]
    bias16 = bias_h[:, :]
    yT16 = yT_h[:, :]

    with tile.TileContext(nc) as tc:
        with (
            tc.tile_pool(name="consts", bufs=1) as consts,
            tc.tile_pool(name="persist", bufs=1) as persist,
            tc.tile_pool(name="wpool", bufs=3) as wpool,
            tc.tile_pool(name="wspool", bufs=3) as wspool,
            tc.tile_pool(name="xpool", bufs=3) as xpool,
            tc.tile_pool(name="stats", bufs=4) as stats,
            tc.tile_pool(name="trash", bufs=2) as trash,
            tc.tile_pool(name="ypool", bufs=3) as ypool,
            tc.tile_pool(name="psum", bufs=4, space="PSUM") as psum,
        ):
            eps_t = consts.tile([P, 1], F32)
            nc.vector.memset(eps_t, EPS)

            # persistent operands
            wsT = persist.tile([P, KT, OUT_F], F16)   # sign(w-mu)^T  [i, o]
            xT = persist.tile([P, KT, B], F16)        # x^T -> r*x^T in place
            r_bcast = persist.tile([P, B], F16)       # r_b rows (all parts)
            nmr_bcast = persist.tile([P, B], F16)     # -(mu*r)_b rows
            stats_cols = persist.tile([P, P], F16)    # cols 0..7 -mu*r, 8..15 r
            statsT = persist.tile([P, P], F16)
            beta_all = persist.tile([P, OT], F32)     # beta col per o-tile
            sb_all = persist.tile([P, OT], F32)       # beta*S col per o-tile
            bias_cols = persist.tile([P, OT], F16)    # bias col per o-tile
            bias32 = persist.tile([P, OT], F32)

            nc.vector.memset(stats_cols, 0.0)

            # bias in column layout [128, OT] (transposed load), cast to f32
            bias_tiles = bias16.rearrange("a (t c) -> (a t) c", c=P)  # [16,128]
            nc.sync.dma_start(out=bias_cols, in_=bias_tiles, transpose=True)
            nc.vector.tensor_copy(out=bias32, in_=bias_cols)

            # ---------------- x pipeline ----------------
            for bt in range(BT):
                xt = xpool.tile([P, IN_F], F16)
                nc.gpsimd.dma_start(out=xt, in_=x16[bt * P:(bt + 1) * P, :])
                st = stats.tile([P, 4, 6], F32)
                for g in range(4):
                    nc.vector.bn_stats(
                        out=st[:, g, :], in_=xt[:, g * 512:(g + 1) * 512])
                mv = stats.tile([P, 2], F32)
                nc.vector.bn_aggr(out=mv, in_=st)
                std = stats.tile([P, 1], F32)
                nc.scalar.activation(
                    out=std, in_=mv[:, 1:2], func=AF.Sqrt, bias=eps_t,
                    scale=1.0)
                r32 = stats.tile([P, 1], F32)
                nc.vector.reciprocal(out=r32, in_=std)
                # -(mu * r) and r columns, fp16
                nc.vector.tensor_scalar(
                    out=stats_cols[:, bt:bt + 1], in0=mv[:, 0:1], scalar1=r32,
                    scalar2=-1.0, op0=MUL, op1=MUL)
                nc.vector.tensor_copy(
                    out=stats_cols[:, 8 + bt:8 + bt + 1], in_=r32)

            # transposed load of x (DRAM -> SBUF via xbar): blocked rows,
            # xT[p, k, b] = x[b, k*128+p]; strictly-2D ops per k block
            for k in range(KT):
                nc.sync.dma_start(
                    out=xT[:, k, :], in_=x16[:, k * P:(k + 1) * P],
                    transpose=True)

            # stats -> row layout + broadcast
            nc.sync.dma_start(out=statsT, in_=stats_cols, transpose=True)
            nmr_rows = nmr_h[0:1, :].rearrange("a (b c) -> (a b) c", c=P)
            r_rows = r_h[0:1, :].rearrange("a (b c) -> (a b) c", c=P)
            nc.sync.dma_start(out=nmr_rows, in_=statsT[0:BT, :])
            nc.sync.dma_start(out=r_rows, in_=statsT[8:8 + BT, :])
            for t, src in ((r_bcast, r_h), (nmr_bcast, nmr_h)):
                s = src[0:1, :]
                bc_ap = bass.AP(
                    tensor=s.tensor, offset=s.offset, ap=[[0, P], [1, B]])
                nc.gpsimd.dma_start(out=t, in_=bc_ap)

            # scale x^T by r (in place)
            for k in range(KT):
                nc.vector.tensor_mul(
                    out=xT[:, k, :], in0=xT[:, k, :], in1=r_bcast)

            # ------------- per-weight-tile pipeline + matmul -------------
            for ot in range(OT):
                wt = wpool.tile([P, IN_F], F32)
                nc.gpsimd.dma_start(out=wt, in_=w[ot * P:(ot + 1) * P, :])

                sumw = stats.tile([P, 1], F32)
                nc.vector.reduce_sum(out=sumw, in_=wt, axis=AXF)
                negmean = stats.tile([P, 1], F32)
                nc.vector.tensor_scalar_mul(
                    out=negmean, in0=sumw, scalar1=-1.0 / IN_F)

                tr = trash.tile([P, IN_F], F16)
                asum = stats.tile([P, 1], F32)
                nc.scalar.activation(
                    out=tr, in_=wt, func=AF.Abs, accum_out=asum)
                nc.vector.tensor_scalar_mul(
                    out=beta_all[:, ot:ot + 1], in0=asum, scalar1=1.0 / IN_F)

                wb = wspool.tile([P, IN_F], F16)
                ssum = stats.tile([P, 1], F32)
                nc.scalar.activation(
                    out=wb, in_=wt, func=AF.Sign, bias=negmean, scale=1.0,
                    accum_out=ssum)
                nc.vector.tensor_mul(
                    out=sb_all[:, ot:ot + 1], in0=ssum,
                    in1=beta_all[:, ot:ot + 1])

                # xbar transposes per (o-tile, k) block, strictly 2D:
                # wsT[p, k, ot*128+f] = sign^T[k*128+p, ot*128+f]
                teng = nc.sync if ot % 2 == 0 else nc.scalar
                for k in range(KT):
                    teng.dma_start(
                        out=wsT[:, k, ot * P:(ot + 1) * P],
                        in_=wb[:, k * P:(k + 1) * P],
                        transpose=True)

                # psum group: out[o-block, b] accumulated over k
                ps = psum.tile([P, B], F32)
                osl = slice(ot * P, (ot + 1) * P)
                for k in range(KT):
                    for bc in range(BC):
                        bsl = slice(bc * NC_CHUNK, (bc + 1) * NC_CHUNK)
                        nc.tensor.matmul(
                            ps[:, bsl], wsT[:, k, osl], xT[:, k, bsl],
                            start=(k == 0), stop=(k == KT - 1))

                # fused epilogue:
                # ct   = nmr_bcast * (beta*S) + bias
                # y^T  = psum * beta + ct
                ct = ypool.tile([P, B], F16, tag="ct")
                nc.vector.tensor_scalar(
                    out=ct, in0=nmr_bcast, scalar1=sb_all[:, ot:ot + 1],
                    scalar2=bias32[:, ot:ot + 1], op0=MUL, op1=ADD)
                ysb = ypool.tile([P, B], F16, tag="ysb")
                nc.vector.scalar_tensor_tensor(
                    out=ysb, in0=ps, scalar=beta_all[:, ot:ot + 1], in1=ct,
                    op0=MUL, op1=ADD)
                nc.sync.dma_start(out=yT16[osl, :], in_=ysb)

    return nc


_NC_CACHE = None
LAST_RESULT = None


def _get_program():
    global _NC_CACHE
    if _NC_CACHE is None:
        nc = _build_program()
        nc.finalize()
        _NC_CACHE = nc
    return _NC_CACHE


def kernel(x: np.ndarray, weight: np.ndarray, bias: np.ndarray) -> np.ndarray:
    global LAST_RESULT
    assert x.shape == (BATCH, IN_F) and weight.shape == (OUT_F, IN_F)

    nc = _get_program()

    x16 = np.ascontiguousarray(x.astype(np.float16))
    w32 = np.ascontiguousarray(weight.astype(np.float32))
    b16 = np.ascontiguousarray(bias.astype(np.float16).reshape(1, OUT_F))

    in_maps = []
    for c in range(N_CORES):
        in_maps.append({
            "x16": np.ascontiguousarray(x16[c * B:(c + 1) * B, :]),
            "w": w32,
            "bias16": b16,
        })

    trace = bool(int(os.environ.get("BITLIN_TRACE", "0")))
    res = run_bass_kernel_spmd(
        nc, in_maps, core_ids=list(range(N_CORES)), trace=trace)
    LAST_RESULT = res

    y = np.concatenate(
        [np.asarray(res.results[c]["yT16"]).T for c in range(N_CORES)],
        axis=0)
    return y.astype(np.float32)
